# revision 1
# baseline (speedup 1.0000x reference)
"""Expert-parallel MoE (Kimi/DeepSeek-V3 style sparse block) on 8 trn2 NeuronCores.

Strategy:
  - Host computes the (tiny) sigmoid gate + group-limited top-2 routing in
    numpy float64 and gathers each expert's tokens into a fixed-capacity,
    transposed batch.
  - Core e runs expert e's SwiGLU FFN over its gathered tokens (capacity
    CAP, in NT-token tiles), plus the shared-expert FFN (split into two
    F=512 halves) over token slice [512*e : 512*(e+1)].  All matmuls run on
    the PE in float32r (full-rate fp32) accumulating in fp32 PSUM.
  - Host scatter-adds the weighted expert outputs and the shared outputs
    back into the full [B,S,D] result.
"""

from contextlib import ExitStack

import numpy as np

import concourse.bacc as bacc
import concourse.tile as tile
import concourse.mybir as mybir
from concourse import bass_utils

# --- model dims (hardcoded per problem spec) ---
B, S, D = 2, 2048, 1024
T = B * S                 # 4096 tokens
E, F = 8, 512             # routed experts / expert intermediate
SH = 1024                 # shared intermediate
TOP_K, N_GROUP, TOPK_GROUP = 2, 4, 2
SCALE = 2.5

N_CORES = 8
P = 128                   # SBUF partitions
NT = 512                  # token tile (matmul moving free dim)
KD = D // P               # 8 contraction tiles for D
KF = F // P               # 4 contraction tiles for F
CAP = 1536                # per-expert token capacity (3 NT-chunks)
NCH = CAP // NT
TSLICE = T // N_CORES     # 512 shared-expert tokens per core

F32 = mybir.dt.float32
MM_DT = mybir.dt.float32r  # full-rate fp32 matmul mode

_CACHE: dict = {}


def _emit(nc):
    """Per-core program: 3 expert-chunk FFN jobs + 2 shared-half FFN jobs."""
    xe = nc.dram_tensor("xe", [D, CAP], MM_DT, kind="ExternalInput").ap()
    xs = nc.dram_tensor("xs", [D, TSLICE], MM_DT, kind="ExternalInput").ap()
    w1t = nc.dram_tensor("w1t", [D, F], MM_DT, kind="ExternalInput").ap()
    w3t = nc.dram_tensor("w3t", [D, F], MM_DT, kind="ExternalInput").ap()
    w2t = nc.dram_tensor("w2t", [F, D], MM_DT, kind="ExternalInput").ap()
    sgt = nc.dram_tensor("sgt", [D, SH], MM_DT, kind="ExternalInput").ap()
    sut = nc.dram_tensor("sut", [D, SH], MM_DT, kind="ExternalInput").ap()
    sdt = nc.dram_tensor("sdt", [SH, D], MM_DT, kind="ExternalInput").ap()
    ye = nc.dram_tensor("ye", [D, CAP], F32, kind="ExternalOutput").ap()
    ys = nc.dram_tensor("ys", [D, TSLICE], F32, kind="ExternalOutput").ap()

    silu = mybir.ActivationFunctionType.Silu

    with tile.TileContext(nc) as tc, ExitStack() as ctx:
        wpool = ctx.enter_context(tc.tile_pool(name="wpool", bufs=2))
        xpool = ctx.enter_context(tc.tile_pool(name="xpool", bufs=2))
        hpool = ctx.enter_context(tc.tile_pool(name="hpool", bufs=2))
        opool = ctx.enter_context(tc.tile_pool(name="opool", bufs=4))
        sopool = ctx.enter_context(tc.tile_pool(name="sopool", bufs=1))
        pspool = ctx.enter_context(tc.tile_pool(name="pspool", bufs=1, space="PSUM"))

        # accumulator for the two shared halves, staged in SBUF
        shared_sb = sopool.tile([P, KD, NT], F32, name="shared_sb")

        def load_wset(w1_d, w3_d, w2_d):
            w1sb = wpool.tile([P, KD, F], MM_DT, name="w1sb", tag="w1")
            nc.sync.dma_start(w1sb[:], w1_d.rearrange("(k p) f -> p k f", p=P))
            w3sb = wpool.tile([P, KD, F], MM_DT, name="w3sb", tag="w3")
            nc.sync.dma_start(w3sb[:], w3_d.rearrange("(k p) f -> p k f", p=P))
            w2sb = wpool.tile([P, KF, D], MM_DT, name="w2sb", tag="w2")
            nc.sync.dma_start(w2sb[:], w2_d.rearrange("(k p) d -> p k d", p=P))
            return w1sb, w3sb, w2sb

        def load_x(x_cols):
            xsb = xpool.tile([P, KD, NT], MM_DT, name="xsb", tag="x")
            nc.sync.dma_start(xsb[:], x_cols.rearrange("(k p) n -> p k n", p=P))
            return xsb

        # 8 PSUM banks, tagged; up-phase uses them as h1/h3 per F-tile,
        # down-phase reuses the same slots for the 8 D-tiles of y.
        ps_tags = [f"ps{i}" for i in range(8)]

        def ffn_job(wset, xsb, mode, out_cols=None):
            w1sb, w3sb, w2sb = wset
            hts = []
            for mf in range(KF):
                h1 = pspool.tile([P, NT], F32, name="h1ps", tag=ps_tags[2 * mf])
                for k in range(KD):
                    nc.tensor.matmul(
                        h1[:],
                        w1sb[:, k, mf * P:(mf + 1) * P],
                        xsb[:, k, :],
                        start=(k == 0), stop=(k == KD - 1),
                    )
                h3 = pspool.tile([P, NT], F32, name="h3ps", tag=ps_tags[2 * mf + 1])
                for k in range(KD):
                    nc.tensor.matmul(
                        h3[:],
                        w3sb[:, k, mf * P:(mf + 1) * P],
                        xsb[:, k, :],
                        start=(k == 0), stop=(k == KD - 1),
                    )
                a = hpool.tile([P, NT], F32, name="asb", tag="silu")
                nc.scalar.activation(a[:], h1[:], silu)
                ht = hpool.tile([P, NT], MM_DT, name="htsb", tag=f"ht{mf}")
                nc.vector.tensor_mul(ht[:], a[:], h3[:])
                hts.append(ht)

            for md in range(KD):
                yps = pspool.tile([P, NT], F32, name="yps", tag=ps_tags[md])
                for kf in range(KF):
                    nc.tensor.matmul(
                        yps[:],
                        w2sb[:, kf, md * P:(md + 1) * P],
                        hts[kf][:],
                        start=(kf == 0), stop=(kf == KF - 1),
                    )
                if mode == "expert":
                    osb = opool.tile([P, NT], F32, name="osb", tag="osb")
                    nc.vector.tensor_copy(osb[:], yps[:])
                    nc.sync.dma_start(
                        ye[md * P:(md + 1) * P, out_cols * NT:(out_cols + 1) * NT],
                        osb[:],
                    )
                elif mode == "shared0":
                    nc.vector.tensor_copy(shared_sb[:, md, :], yps[:])
                else:  # shared1: accumulate and emit
                    nc.vector.tensor_add(shared_sb[:, md, :], shared_sb[:, md, :], yps[:])
                    nc.sync.dma_start(
                        ys[md * P:(md + 1) * P, :], shared_sb[:, md, :]
                    )

        wexp = load_wset(w1t, w3t, w2t)
        for c in range(NCH):
            xsb = load_x(xe[:, c * NT:(c + 1) * NT])
            ffn_job(wexp, xsb, "expert", out_cols=c)

        xss = load_x(xs)
        wsh0 = load_wset(sgt[:, 0:F], sut[:, 0:F], sdt[0:F, :])
        ffn_job(wsh0, xss, "shared0")
        wsh1 = load_wset(sgt[:, F:SH], sut[:, F:SH], sdt[F:SH, :])
        ffn_job(wsh1, xss, "shared1")


def _get_nc():
    if "nc" not in _CACHE:
        nc = bacc.Bacc("TRN2", target_bir_lowering=False, debug=False,
                       num_devices=N_CORES)
        _emit(nc)
        nc.compile()
        _CACHE["nc"] = nc
    return _CACHE["nc"]


def _gate_numpy(x2d):
    """Replicates reference _moe_gate in float64 (routing-stable)."""
    xl = x2d.astype(np.float64)
    logits = xl @ _CACHE["gw64"].T
    scores = 1.0 / (1.0 + np.exp(-logits))
    sc = scores + _CACHE["gb64"][None, :]
    grp = sc.reshape(T, N_GROUP, E // N_GROUP)
    group_scores = np.sort(grp, axis=-1)[:, :, -2:].sum(-1)
    gidx = np.argsort(-group_scores, axis=-1, kind="stable")[:, :TOPK_GROUP]
    gmask = np.zeros((T, N_GROUP), bool)
    gmask[np.arange(T)[:, None], gidx] = True
    smask = np.repeat(gmask, E // N_GROUP, axis=1)
    tmp = np.where(smask, sc, 0.0)
    tidx = np.argsort(-tmp, axis=-1, kind="stable")[:, :TOP_K]
    tw = np.take_along_axis(scores, tidx, axis=1)
    tw = tw / (tw.sum(-1, keepdims=True) + 1e-20)
    return tidx, (tw * SCALE).astype(np.float32)


def _ffn_host(x, w1e, w2e, w3e):
    """Host fallback for capacity-overflow tokens (rare)."""
    h = x @ w1e.T
    h = (h / (1.0 + np.exp(-h))) * (x @ w3e.T)
    return h @ w2e.T


def kernel(hidden_states, gate_w, gate_bias, w1, w2, w3,
           shared_gate_w, shared_up_w, shared_down_w):
    hidden_states = np.ascontiguousarray(np.asarray(hidden_states, np.float32))
    gate_w = np.asarray(gate_w, np.float32)
    gate_bias = np.asarray(gate_bias, np.float32)
    w1 = np.asarray(w1, np.float32)
    w2 = np.asarray(w2, np.float32)
    w3 = np.asarray(w3, np.float32)
    shared_gate_w = np.asarray(shared_gate_w, np.float32)
    shared_up_w = np.asarray(shared_up_w, np.float32)
    shared_down_w = np.asarray(shared_down_w, np.float32)

    _CACHE["gw64"] = gate_w.astype(np.float64)
    _CACHE["gb64"] = gate_bias.astype(np.float64)

    x2d = hidden_states.reshape(T, D)
    tidx, tw = _gate_numpy(x2d)

    # per-expert gather lists
    ew_cols = [np.nonzero(tidx == e) for e in range(E)]  # (rows, which-slot)
    in_maps = []
    idx_list, wt_list, n_list = [], [], []
    overflow = []
    x2dT = np.ascontiguousarray(x2d.T)  # [D, T] — column gathers are cheap
    for e in range(E):
        rows, slots = ew_cols[e]
        n = len(rows)
        if n > CAP:
            overflow.append((e, rows[CAP:], slots[CAP:]))
            rows, slots = rows[:CAP], slots[:CAP]
            n = CAP
        idx_list.append(rows)
        wt_list.append(tw[rows, slots])
        n_list.append(n)
        xeT = np.zeros((D, CAP), np.float32)
        xeT[:, :n] = x2dT[:, rows]
        sl = slice(e * TSLICE, (e + 1) * TSLICE)
        in_maps.append({
            "xe": xeT,
            "xs": np.ascontiguousarray(x2dT[:, sl]),
            "w1t": np.ascontiguousarray(w1[e].T),
            "w3t": np.ascontiguousarray(w3[e].T),
            "w2t": np.ascontiguousarray(w2[e].T),
            "sgt": np.ascontiguousarray(shared_gate_w.T),
            "sut": np.ascontiguousarray(shared_up_w.T),
            "sdt": np.ascontiguousarray(shared_down_w.T),
        })

    nc = _get_nc()
    res = bass_utils.run_bass_kernel_spmd(
        nc, in_maps, core_ids=list(range(N_CORES))
    )
    _CACHE["last_res"] = res

    y = np.zeros((T, D), np.float32)
    for e in range(E):
        n = n_list[e]
        out = res.results[e]
        if n:
            ye = out["ye"][:, :n].T  # [n, D]
            y[idx_list[e]] += wt_list[e][:, None] * ye
        sl = slice(e * TSLICE, (e + 1) * TSLICE)
        y[sl] += out["ys"].T
    for e, rows, slots in overflow:
        y[rows] += tw[rows, slots][:, None] * _ffn_host(x2d[rows], w1[e], w2[e], w3[e])

    return y.reshape(B, S, D)



# revision 4
# speedup vs baseline: 1.2981x; 1.2981x over previous
"""Expert-parallel MoE (Kimi/DeepSeek-V3 style sparse block) on 8 trn2 NeuronCores.

Strategy (v2 — balanced, bf16):
  - Host computes the sigmoid gate + group-limited top-2 routing in numpy
    float64, then packs a *balanced* per-core schedule of five matmul chunks:
      [own-expert 512] [own-expert 512] [overflow-frag 128] [shared 512] [shared 512]
    Core c owns expert c's first 1024 tokens; per-expert overflow beyond 1024
    is split into <=128-token single-expert fragments bin-packed across the
    8 cores' fragment slots.  The shared FFN (SH=1024) is split into two
    F=512 halves; cores 0-3 run half 0 and cores 4-7 run half 1, each over a
    distinct 1024-token slice, summed on the host.
  - All weights/activations ship as bf16 in the exact SBUF layout (contiguous
    8KB-per-partition DMA lines); matmuls accumulate in fp32 PSUM; outputs
    return as bf16 and are combined on the host in fp32.
"""

from contextlib import ExitStack

import numpy as np
import ml_dtypes

import concourse.bacc as bacc
import concourse.tile as tile
import concourse.mybir as mybir
from concourse import bass_utils

# --- model dims (hardcoded per problem spec) ---
B, S, D = 2, 2048, 1024
T = B * S                 # 4096 tokens
E, F = 8, 512             # routed experts / expert intermediate
SH = 1024                 # shared intermediate
TOP_K, N_GROUP, TOPK_GROUP = 2, 4, 2
SCALE = 2.5

N_CORES = 8
P = 128                   # SBUF partitions
NT = 512                  # full token chunk (matmul moving free dim)
NB = 128                  # overflow-fragment chunk
KD = D // P               # 8 contraction tiles for D
KF = F // P               # 4 contraction tiles for F
ACAP = 2 * NT             # own-expert capacity per core (2 chunks)
SSLICE = 2 * NT           # shared tokens per core (2 chunks)

F32 = mybir.dt.float32
BF16 = mybir.dt.bfloat16
NPBF16 = ml_dtypes.bfloat16

_CACHE: dict = {}


def _emit(nc):
    """Per-core program: 2 own-expert chunks + 1 fragment chunk + 2 shared chunks."""
    wa1 = nc.dram_tensor("wa1", [P, KD, F], BF16, kind="ExternalInput").ap()
    wa3 = nc.dram_tensor("wa3", [P, KD, F], BF16, kind="ExternalInput").ap()
    wa2 = nc.dram_tensor("wa2", [P, KF, D], BF16, kind="ExternalInput").ap()
    wb1 = nc.dram_tensor("wb1", [P, KD, F], BF16, kind="ExternalInput").ap()
    wb3 = nc.dram_tensor("wb3", [P, KD, F], BF16, kind="ExternalInput").ap()
    wb2 = nc.dram_tensor("wb2", [P, KF, D], BF16, kind="ExternalInput").ap()
    ws1 = nc.dram_tensor("ws1", [P, KD, F], BF16, kind="ExternalInput").ap()
    ws3 = nc.dram_tensor("ws3", [P, KD, F], BF16, kind="ExternalInput").ap()
    ws2 = nc.dram_tensor("ws2", [P, KF, D], BF16, kind="ExternalInput").ap()
    xa = nc.dram_tensor("xa", [2, P, KD, NT], BF16, kind="ExternalInput").ap()
    xb = nc.dram_tensor("xb", [P, KD, NB], BF16, kind="ExternalInput").ap()
    xs = nc.dram_tensor("xs", [2, P, KD, NT], BF16, kind="ExternalInput").ap()
    ya = nc.dram_tensor("ya", [2, KD, P, NT], BF16, kind="ExternalOutput").ap()
    yb = nc.dram_tensor("yb", [KD, P, NB], BF16, kind="ExternalOutput").ap()
    ys = nc.dram_tensor("ys", [2, KD, P, NT], BF16, kind="ExternalOutput").ap()

    silu = mybir.ActivationFunctionType.Silu

    with tile.TileContext(nc) as tc, ExitStack() as ctx:
        wpool = ctx.enter_context(tc.tile_pool(name="wpool", bufs=2))
        xpool = ctx.enter_context(tc.tile_pool(name="xpool", bufs=2))
        hpool = ctx.enter_context(tc.tile_pool(name="hpool", bufs=2))
        opool = ctx.enter_context(tc.tile_pool(name="opool", bufs=4))
        pspool = ctx.enter_context(tc.tile_pool(name="pspool", bufs=1, space="PSUM"))

        def load_wset(w1_d, w3_d, w2_d):
            w1sb = wpool.tile([P, KD, F], BF16, name="w1sb", tag="w1")
            nc.sync.dma_start(w1sb[:], w1_d)
            w3sb = wpool.tile([P, KD, F], BF16, name="w3sb", tag="w3")
            nc.sync.dma_start(w3sb[:], w3_d)
            w2sb = wpool.tile([P, KF, D], BF16, name="w2sb", tag="w2")
            nc.sync.dma_start(w2sb[:], w2_d)
            return w1sb, w3sb, w2sb

        def load_x(x_d, n):
            xsb = xpool.tile([P, KD, n], BF16, name="xsb", tag="x")
            nc.sync.dma_start(xsb[:], x_d)
            return xsb

        ps_tags = [f"ps{i}" for i in range(8)]

        def ffn_job(wset, xsb, n, y_d):
            w1sb, w3sb, w2sb = wset
            hts = []
            for mf in range(KF):
                h1 = pspool.tile([P, n], F32, name="h1ps", tag=ps_tags[2 * mf])
                for k in range(KD):
                    nc.tensor.matmul(
                        h1[:],
                        w1sb[:, k, mf * P:(mf + 1) * P],
                        xsb[:, k, :],
                        start=(k == 0), stop=(k == KD - 1),
                    )
                h3 = pspool.tile([P, n], F32, name="h3ps", tag=ps_tags[2 * mf + 1])
                for k in range(KD):
                    nc.tensor.matmul(
                        h3[:],
                        w3sb[:, k, mf * P:(mf + 1) * P],
                        xsb[:, k, :],
                        start=(k == 0), stop=(k == KD - 1),
                    )
                a = hpool.tile([P, n], F32, name="asb", tag="silu")
                nc.scalar.activation(a[:], h1[:], silu)
                ht = hpool.tile([P, n], BF16, name="htsb", tag=f"ht{mf}")
                nc.vector.tensor_mul(ht[:], a[:], h3[:])
                hts.append(ht)

            for md in range(KD):
                yps = pspool.tile([P, n], F32, name="yps", tag=ps_tags[md])
                for kf in range(KF):
                    nc.tensor.matmul(
                        yps[:],
                        w2sb[:, kf, md * P:(md + 1) * P],
                        hts[kf][:],
                        start=(kf == 0), stop=(kf == KF - 1),
                    )
                osb = opool.tile([P, n], BF16, name="osb", tag="osb")
                nc.vector.tensor_copy(osb[:], yps[:])
                nc.sync.dma_start(y_d[md], osb[:])

        wA = load_wset(wa1, wa3, wa2)
        for c in range(2):
            xsb = load_x(xa[c], NT)
            ffn_job(wA, xsb, NT, ya[c])

        wB = load_wset(wb1, wb3, wb2)
        xbb = load_x(xb, NB)
        ffn_job(wB, xbb, NB, yb)

        wS = load_wset(ws1, ws3, ws2)
        for c in range(2):
            xsb = load_x(xs[c], NT)
            ffn_job(wS, xsb, NT, ys[c])


def _get_nc():
    if "nc" not in _CACHE:
        nc = bacc.Bacc("TRN2", target_bir_lowering=False, debug=False,
                       num_devices=N_CORES)
        _emit(nc)
        nc.compile()
        _CACHE["nc"] = nc
    return _CACHE["nc"]


def _gate_numpy(x2d):
    """Replicates reference _moe_gate in float64 (routing-stable)."""
    xl = x2d.astype(np.float64)
    logits = xl @ _CACHE["gw64"].T
    scores = 1.0 / (1.0 + np.exp(-logits))
    sc = scores + _CACHE["gb64"][None, :]
    grp = sc.reshape(T, N_GROUP, E // N_GROUP)
    group_scores = np.sort(grp, axis=-1)[:, :, -2:].sum(-1)
    gidx = np.argsort(-group_scores, axis=-1, kind="stable")[:, :TOPK_GROUP]
    gmask = np.zeros((T, N_GROUP), bool)
    gmask[np.arange(T)[:, None], gidx] = True
    smask = np.repeat(gmask, E // N_GROUP, axis=1)
    tmp = np.where(smask, sc, 0.0)
    tidx = np.argsort(-tmp, axis=-1, kind="stable")[:, :TOP_K]
    tw = np.take_along_axis(scores, tidx, axis=1)
    tw = tw / (tw.sum(-1, keepdims=True) + 1e-20)
    return tidx, (tw * SCALE).astype(np.float32)


def _ffn_host(x, w1e, w2e, w3e):
    """Host fallback for tokens that don't fit the fragment slots (rare)."""
    h = x @ w1e.T
    h = (h / (1.0 + np.exp(-h))) * (x @ w3e.T)
    return h @ w2e.T


def _wlay_up(w):
    """[F, D] row-major -> [P, KD, F] bf16 (SBUF layout, d = k*P + p)."""
    return np.ascontiguousarray(
        w.T.reshape(KD, P, F).transpose(1, 0, 2).astype(NPBF16))


def _wlay_down(w):
    """[D, F] row-major -> [P, KF, D] bf16 (SBUF layout, f = k*P + p)."""
    return np.ascontiguousarray(
        w.T.reshape(KF, P, D).transpose(1, 0, 2).astype(NPBF16))


def _xlay(xrows, nchunk, clen=NT):
    """[n<=nchunk*clen, D] tokens -> [nchunk, P, KD, clen] bf16, zero-padded."""
    n = xrows.shape[0]
    full = np.zeros((nchunk * clen, D), np.float32)
    full[:n] = xrows
    return np.ascontiguousarray(
        full.reshape(nchunk, clen, KD, P).transpose(0, 3, 2, 1).astype(NPBF16))


def _ylay(y):
    """[KD, P, n] bf16 -> [n, D] fp32."""
    kd, p, n = y.shape
    return y.astype(np.float32).reshape(kd * p, n).T


def kernel(hidden_states, gate_w, gate_bias, w1, w2, w3,
           shared_gate_w, shared_up_w, shared_down_w):
    hidden_states = np.ascontiguousarray(np.asarray(hidden_states, np.float32))
    gate_w = np.asarray(gate_w, np.float32)
    gate_bias = np.asarray(gate_bias, np.float32)
    w1 = np.asarray(w1, np.float32)
    w2 = np.asarray(w2, np.float32)
    w3 = np.asarray(w3, np.float32)
    shared_gate_w = np.asarray(shared_gate_w, np.float32)
    shared_up_w = np.asarray(shared_up_w, np.float32)
    shared_down_w = np.asarray(shared_down_w, np.float32)

    _CACHE["gw64"] = gate_w.astype(np.float64)
    _CACHE["gb64"] = gate_bias.astype(np.float64)

    x2d = hidden_states.reshape(T, D)
    tidx, tw = _gate_numpy(x2d)

    # --- per-expert token lists ---
    rows_e, wts_e = [], []
    for e in range(E):
        rows, slots = np.nonzero(tidx == e)
        rows_e.append(rows)
        wts_e.append(tw[rows, slots])

    # --- schedule: core c owns expert c's first ACAP tokens ---
    a_rows = [rows_e[c][:ACAP] for c in range(N_CORES)]
    a_wts = [wts_e[c][:ACAP] for c in range(N_CORES)]

    # overflow fragments: single-expert pieces of <=NB tokens, one per core slot
    frag_rows = [np.empty(0, np.int64)] * N_CORES
    frag_wts = [np.empty(0, np.float32)] * N_CORES
    frag_expert = [-1] * N_CORES
    overflow_host = []          # (expert, rows, wts) for host fallback
    slot = 0
    for e in range(E):
        left_r, left_w = rows_e[e][ACAP:], wts_e[e][ACAP:]
        pos = 0
        while pos < len(left_r):
            if slot >= N_CORES:
                overflow_host.append((e, left_r[pos:], left_w[pos:]))
                break
            take = min(NB, len(left_r) - pos)
            frag_rows[slot] = left_r[pos:pos + take]
            frag_wts[slot] = left_w[pos:pos + take]
            frag_expert[slot] = e
            slot += 1
            pos += take

    # --- build per-core inputs ---
    wup = [_wlay_up(w1[e]) for e in range(E)]
    wgt = [_wlay_up(w3[e]) for e in range(E)]
    wdn = [_wlay_down(w2[e]) for e in range(E)]
    sh1 = [_wlay_up(shared_gate_w[h * F:(h + 1) * F]) for h in range(2)]
    sh3 = [_wlay_up(shared_up_w[h * F:(h + 1) * F]) for h in range(2)]
    sh2 = [_wlay_down(shared_down_w[:, h * F:(h + 1) * F]) for h in range(2)]

    in_maps = []
    for c in range(N_CORES):
        eb = frag_expert[c] if frag_expert[c] >= 0 else c
        h = c // 4
        ssl = slice((c % 4) * SSLICE, (c % 4 + 1) * SSLICE)
        in_maps.append({
            "wa1": wup[c], "wa3": wgt[c], "wa2": wdn[c],
            "wb1": wup[eb], "wb3": wgt[eb], "wb2": wdn[eb],
            "ws1": sh1[h], "ws3": sh3[h], "ws2": sh2[h],
            "xa": _xlay(x2d[a_rows[c]], 2),
            "xb": _xlay(x2d[frag_rows[c]], 1, NB)[0],
            "xs": _xlay(x2d[ssl], 2),
        })

    nc = _get_nc()
    res = bass_utils.run_bass_kernel_spmd(
        nc, in_maps, core_ids=list(range(N_CORES))
    )
    _CACHE["last_res"] = res

    y = np.zeros((T, D), np.float32)
    for c in range(N_CORES):
        out = res.results[c]
        ya = out["ya"]          # [2, KD, P, NT] bf16
        n = len(a_rows[c])
        yaf = np.concatenate([_ylay(ya[0]), _ylay(ya[1])], axis=0)[:n]
        y[a_rows[c]] += a_wts[c][:, None] * yaf
        nf = len(frag_rows[c])
        if nf:
            ybf = _ylay(out["yb"])[:nf]
            y[frag_rows[c]] += frag_wts[c][:, None] * ybf
        ssl = slice((c % 4) * SSLICE, (c % 4 + 1) * SSLICE)
        ysx = out["ys"]
        y[ssl] += np.concatenate([_ylay(ysx[0]), _ylay(ysx[1])], axis=0)
    for e, rows, wts in overflow_host:
        y[rows] += wts[:, None] * _ffn_host(x2d[rows], w1[e], w2[e], w3[e])

    return y.reshape(B, S, D)


# revision 5
# speedup vs baseline: 1.3245x; 1.0203x over previous
"""Expert-parallel MoE (Kimi/DeepSeek-V3 style sparse block) on 8 trn2 NeuronCores.

Strategy (v3 — balanced, bf16, overlap-tuned):
  - Host computes the sigmoid gate + group-limited top-2 routing in numpy
    float64, then packs a *balanced* per-core schedule of five matmul chunks:
      [own-expert 512]x2  [shared 512]x2  [overflow-frag 128]
    Core c owns expert c's first 1024 tokens; per-expert overflow beyond 1024
    is split into <=128-token single-expert fragments bin-packed across the
    8 cores' fragment slots.  The shared FFN (SH=1024) is split into two
    F=512 halves; cores 0-3 run half 0 and cores 4-7 run half 1, each over a
    distinct 1024-token slice, summed on the host.
  - All weights/activations ship as bf16 in the exact SBUF layout (contiguous
    per-partition DMA lines); matmuls accumulate in fp32 PSUM; outputs return
    bf16 and are combined on the host in fp32.
  - Overlap tuning: every SBUF tile has a dedicated buffer so DMA issues
    never stall; the first chunk's weight/x loads are split in half so the
    first matmul starts as early as possible; eight warm-up matmuls on a
    memset tile run during the load window to exit the HAM cold state; the
    small fragment chunk runs last so the drain tail is short.
"""

from contextlib import ExitStack

import numpy as np
import ml_dtypes

import concourse.bacc as bacc
import concourse.tile as tile
import concourse.mybir as mybir
from concourse import bass_utils

# --- model dims (hardcoded per problem spec) ---
B, S, D = 2, 2048, 1024
T = B * S                 # 4096 tokens
E, F = 8, 512             # routed experts / expert intermediate
SH = 1024                 # shared intermediate
TOP_K, N_GROUP, TOPK_GROUP = 2, 4, 2
SCALE = 2.5

N_CORES = 8
P = 128                   # SBUF partitions
NT = 512                  # full token chunk (matmul moving free dim)
NB = 128                  # overflow-fragment chunk
KD = D // P               # 8 contraction tiles for D
KF = F // P               # 4 contraction tiles for F
ACAP = 2 * NT             # own-expert capacity per core (2 chunks)
SSLICE = 2 * NT           # shared tokens per core (2 chunks)

F32 = mybir.dt.float32
BF16 = mybir.dt.bfloat16
NPBF16 = ml_dtypes.bfloat16

_CACHE: dict = {}


def _emit(nc):
    """Per-core program: 2 own-expert chunks + 2 shared chunks + 1 fragment."""
    wa13 = nc.dram_tensor("wa13", [P, KD, 2 * F], BF16, kind="ExternalInput").ap()
    wa2 = nc.dram_tensor("wa2", [P, KF, D], BF16, kind="ExternalInput").ap()
    ws13 = nc.dram_tensor("ws13", [P, KD, 2 * F], BF16, kind="ExternalInput").ap()
    ws2 = nc.dram_tensor("ws2", [P, KF, D], BF16, kind="ExternalInput").ap()
    wb13 = nc.dram_tensor("wb13", [P, KD, 2 * F], BF16, kind="ExternalInput").ap()
    wb2 = nc.dram_tensor("wb2", [P, KF, D], BF16, kind="ExternalInput").ap()
    xa = nc.dram_tensor("xa", [2, P, KD, NT], BF16, kind="ExternalInput").ap()
    xs = nc.dram_tensor("xs", [2, P, KD, NT], BF16, kind="ExternalInput").ap()
    xb = nc.dram_tensor("xb", [P, KD, NB], BF16, kind="ExternalInput").ap()
    ya = nc.dram_tensor("ya", [2, P, KD, NT], BF16, kind="ExternalOutput").ap()
    ys = nc.dram_tensor("ys", [2, P, KD, NT], BF16, kind="ExternalOutput").ap()
    yb = nc.dram_tensor("yb", [KD, P, NB], BF16, kind="ExternalOutput").ap()

    silu = mybir.ActivationFunctionType.Silu

    with tile.TileContext(nc) as tc, ExitStack() as ctx:
        pool = ctx.enter_context(tc.tile_pool(name="sb", bufs=1))
        pspool = ctx.enter_context(tc.tile_pool(name="ps", bufs=1, space="PSUM"))

        ps_tags = [f"ps{i}" for i in range(8)]

        # --- HAM warm-up: ~3.4us of matmuls on a zero tile, no DMA needed ---
        warm = pool.tile([P, NT], BF16, name="warm", tag="warm")
        nc.any.memset(warm[:], 0)
        wps = pspool.tile([P, NT], F32, name="wps", tag=ps_tags[0])
        for _ in range(8):
            nc.tensor.matmul(wps[:], warm[:, 0:P], warm[:], start=True, stop=True)

        # --- loads: every tile has its own buffer; issue order = priority ---
        wa13lo = pool.tile([P, 4, 2 * F], BF16, name="wa13lo", tag="wa13lo")
        nc.sync.dma_start(wa13lo[:], wa13[:, 0:4])
        xa0lo = pool.tile([P, 4, NT], BF16, name="xa0lo", tag="xa0lo")
        nc.sync.dma_start(xa0lo[:], xa[0][:, 0:4])
        wa13hi = pool.tile([P, 4, 2 * F], BF16, name="wa13hi", tag="wa13hi")
        nc.sync.dma_start(wa13hi[:], wa13[:, 4:8])
        xa0hi = pool.tile([P, 4, NT], BF16, name="xa0hi", tag="xa0hi")
        nc.sync.dma_start(xa0hi[:], xa[0][:, 4:8])
        wa2sb = pool.tile([P, KF, D], BF16, name="wa2sb", tag="wa2")
        nc.sync.dma_start(wa2sb[:], wa2)
        xa1sb = pool.tile([P, KD, NT], BF16, name="xa1sb", tag="xa1")
        nc.sync.dma_start(xa1sb[:], xa[1])
        ws13sb = pool.tile([P, KD, 2 * F], BF16, name="ws13sb", tag="ws13")
        nc.sync.dma_start(ws13sb[:], ws13)
        xs0sb = pool.tile([P, KD, NT], BF16, name="xs0sb", tag="xs0")
        nc.sync.dma_start(xs0sb[:], xs[0])
        ws2sb = pool.tile([P, KF, D], BF16, name="ws2sb", tag="ws2")
        nc.sync.dma_start(ws2sb[:], ws2)
        xs1sb = pool.tile([P, KD, NT], BF16, name="xs1sb", tag="xs1")
        nc.sync.dma_start(xs1sb[:], xs[1])
        wb13sb = pool.tile([P, KD, 2 * F], BF16, name="wb13sb", tag="wb13")
        nc.sync.dma_start(wb13sb[:], wb13)
        xbsb = pool.tile([P, KD, NB], BF16, name="xbsb", tag="xb")
        nc.sync.dma_start(xbsb[:], xb)
        wb2sb = pool.tile([P, KF, D], BF16, name="wb2sb", tag="wb2")
        nc.sync.dma_start(wb2sb[:], wb2)

        def ffn_job(w13_at, w2sb, x_at, n, ci, y_d=None, yb_d=None):
            """One chunk: x [D, n] -> SwiGLU FFN -> y [D, n].

            w13_at(k) -> [P, 2F] weight slice for contraction tile k.
            x_at(k)   -> [P, n] activation slice for contraction tile k.
            """
            hts = []
            for mf in range(KF):
                h1 = pspool.tile([P, n], F32, name="h1ps", tag=ps_tags[2 * mf])
                for k in range(KD):
                    nc.tensor.matmul(
                        h1[:], w13_at(k)[:, mf * P:(mf + 1) * P], x_at(k),
                        start=(k == 0), stop=(k == KD - 1),
                    )
                h3 = pspool.tile([P, n], F32, name="h3ps", tag=ps_tags[2 * mf + 1])
                for k in range(KD):
                    nc.tensor.matmul(
                        h3[:], w13_at(k)[:, F + mf * P:F + (mf + 1) * P], x_at(k),
                        start=(k == 0), stop=(k == KD - 1),
                    )
                a = pool.tile([P, n], F32, name="asb", tag=f"a{mf % 2}")
                nc.scalar.activation(a[:], h1[:], silu)
                ht = pool.tile([P, n], BF16, name="htsb", tag=f"ht{mf}")
                nc.vector.tensor_mul(ht[:], a[:], h3[:])
                hts.append(ht)

            osb = pool.tile([P, KD, n], BF16, name="osb", tag=f"osb{ci % 2}")
            for md in range(KD):
                yps = pspool.tile([P, n], F32, name="yps", tag=ps_tags[md])
                for kf in range(KF):
                    nc.tensor.matmul(
                        yps[:], w2sb[:, kf, md * P:(md + 1) * P], hts[kf][:],
                        start=(kf == 0), stop=(kf == KF - 1),
                    )
                nc.vector.tensor_copy(osb[:, md, :], yps[:])
                if yb_d is not None:
                    nc.sync.dma_start(yb_d[md], osb[:, md, :])
            if y_d is not None:
                nc.sync.dma_start(y_d, osb[:])

        # job 0: expert chunk 0 (split tiles for early start)
        ffn_job(lambda k: (wa13lo if k < 4 else wa13hi)[:, k % 4],
                wa2sb,
                lambda k: (xa0lo if k < 4 else xa0hi)[:, k % 4],
                NT, 0, y_d=ya[0])
        # job 1: expert chunk 1
        ffn_job(lambda k: wa13lo[:, k] if k < 4 else wa13hi[:, k - 4],
                wa2sb, lambda k: xa1sb[:, k], NT, 1, y_d=ya[1])
        # jobs 2-3: shared chunks
        ffn_job(lambda k: ws13sb[:, k], ws2sb, lambda k: xs0sb[:, k],
                NT, 2, y_d=ys[0])
        ffn_job(lambda k: ws13sb[:, k], ws2sb, lambda k: xs1sb[:, k],
                NT, 3, y_d=ys[1])
        # job 4: overflow fragment (small, short drain tail)
        ffn_job(lambda k: wb13sb[:, k], wb2sb, lambda k: xbsb[:, k],
                NB, 4, yb_d=yb)


def _get_nc():
    if "nc" not in _CACHE:
        nc = bacc.Bacc("TRN2", target_bir_lowering=False, debug=False,
                       num_devices=N_CORES)
        _emit(nc)
        nc.compile()
        _CACHE["nc"] = nc
    return _CACHE["nc"]


def _gate_numpy(x2d):
    """Replicates reference _moe_gate in float64 (routing-stable)."""
    xl = x2d.astype(np.float64)
    logits = xl @ _CACHE["gw64"].T
    scores = 1.0 / (1.0 + np.exp(-logits))
    sc = scores + _CACHE["gb64"][None, :]
    grp = sc.reshape(T, N_GROUP, E // N_GROUP)
    group_scores = np.sort(grp, axis=-1)[:, :, -2:].sum(-1)
    gidx = np.argsort(-group_scores, axis=-1, kind="stable")[:, :TOPK_GROUP]
    gmask = np.zeros((T, N_GROUP), bool)
    gmask[np.arange(T)[:, None], gidx] = True
    smask = np.repeat(gmask, E // N_GROUP, axis=1)
    tmp = np.where(smask, sc, 0.0)
    tidx = np.argsort(-tmp, axis=-1, kind="stable")[:, :TOP_K]
    tw = np.take_along_axis(scores, tidx, axis=1)
    tw = tw / (tw.sum(-1, keepdims=True) + 1e-20)
    return tidx, (tw * SCALE).astype(np.float32)


def _ffn_host(x, w1e, w2e, w3e):
    """Host fallback for tokens that don't fit the fragment slots (rare)."""
    h = x @ w1e.T
    h = (h / (1.0 + np.exp(-h))) * (x @ w3e.T)
    return h @ w2e.T


def _wlay_up(w):
    """[F, D] row-major -> [P, KD, F] bf16 (SBUF layout, d = k*P + p)."""
    return w.T.reshape(KD, P, F).transpose(1, 0, 2).astype(NPBF16)


def _wlay13(wg, wu):
    """gate/up [F, D] pair -> combined [P, KD, 2F] bf16."""
    return np.ascontiguousarray(
        np.concatenate([_wlay_up(wg), _wlay_up(wu)], axis=2))


def _wlay_down(w):
    """[D, F] row-major -> [P, KF, D] bf16 (SBUF layout, f = k*P + p)."""
    return np.ascontiguousarray(
        w.T.reshape(KF, P, D).transpose(1, 0, 2).astype(NPBF16))


def _xlay(xrows, nchunk, clen=NT):
    """[n<=nchunk*clen, D] tokens -> [nchunk, P, KD, clen] bf16, zero-padded."""
    n = xrows.shape[0]
    full = np.zeros((nchunk * clen, D), np.float32)
    full[:n] = xrows
    return np.ascontiguousarray(
        full.reshape(nchunk, clen, KD, P).transpose(0, 3, 2, 1).astype(NPBF16))


def _ylay(y):
    """[P, KD, n] bf16 -> [n, D] fp32."""
    p, kd, n = y.shape
    return y.astype(np.float32).transpose(2, 1, 0).reshape(n, kd * p)


def _ylay_b(y):
    """[KD, P, n] bf16 -> [n, D] fp32."""
    kd, p, n = y.shape
    return y.astype(np.float32).reshape(kd * p, n).T


def kernel(hidden_states, gate_w, gate_bias, w1, w2, w3,
           shared_gate_w, shared_up_w, shared_down_w):
    hidden_states = np.ascontiguousarray(np.asarray(hidden_states, np.float32))
    gate_w = np.asarray(gate_w, np.float32)
    gate_bias = np.asarray(gate_bias, np.float32)
    w1 = np.asarray(w1, np.float32)
    w2 = np.asarray(w2, np.float32)
    w3 = np.asarray(w3, np.float32)
    shared_gate_w = np.asarray(shared_gate_w, np.float32)
    shared_up_w = np.asarray(shared_up_w, np.float32)
    shared_down_w = np.asarray(shared_down_w, np.float32)

    _CACHE["gw64"] = gate_w.astype(np.float64)
    _CACHE["gb64"] = gate_bias.astype(np.float64)

    x2d = hidden_states.reshape(T, D)
    tidx, tw = _gate_numpy(x2d)

    # --- per-expert token lists ---
    rows_e, wts_e = [], []
    for e in range(E):
        rows, slots = np.nonzero(tidx == e)
        rows_e.append(rows)
        wts_e.append(tw[rows, slots])

    # --- schedule: core c owns expert c's first ACAP tokens ---
    a_rows = [rows_e[c][:ACAP] for c in range(N_CORES)]
    a_wts = [wts_e[c][:ACAP] for c in range(N_CORES)]

    # overflow fragments: single-expert pieces of <=NB tokens, one per core slot
    frag_rows = [np.empty(0, np.int64)] * N_CORES
    frag_wts = [np.empty(0, np.float32)] * N_CORES
    frag_expert = [-1] * N_CORES
    overflow_host = []          # (expert, rows, wts) for host fallback
    slot = 0
    for e in range(E):
        left_r, left_w = rows_e[e][ACAP:], wts_e[e][ACAP:]
        pos = 0
        while pos < len(left_r):
            if slot >= N_CORES:
                overflow_host.append((e, left_r[pos:], left_w[pos:]))
                break
            take = min(NB, len(left_r) - pos)
            frag_rows[slot] = left_r[pos:pos + take]
            frag_wts[slot] = left_w[pos:pos + take]
            frag_expert[slot] = e
            slot += 1
            pos += take

    # --- build per-core inputs ---
    w13 = [_wlay13(w1[e], w3[e]) for e in range(E)]
    wdn = [_wlay_down(w2[e]) for e in range(E)]
    s13 = [_wlay13(shared_gate_w[h * F:(h + 1) * F],
                   shared_up_w[h * F:(h + 1) * F]) for h in range(2)]
    s2 = [_wlay_down(shared_down_w[:, h * F:(h + 1) * F]) for h in range(2)]

    in_maps = []
    for c in range(N_CORES):
        eb = frag_expert[c] if frag_expert[c] >= 0 else c
        h = c // 4
        ssl = slice((c % 4) * SSLICE, (c % 4 + 1) * SSLICE)
        in_maps.append({
            "wa13": w13[c], "wa2": wdn[c],
            "wb13": w13[eb], "wb2": wdn[eb],
            "ws13": s13[h], "ws2": s2[h],
            "xa": _xlay(x2d[a_rows[c]], 2),
            "xb": _xlay(x2d[frag_rows[c]], 1, NB)[0],
            "xs": _xlay(x2d[ssl], 2),
        })

    nc = _get_nc()
    res = bass_utils.run_bass_kernel_spmd(
        nc, in_maps, core_ids=list(range(N_CORES))
    )
    _CACHE["last_res"] = res

    y = np.zeros((T, D), np.float32)
    for c in range(N_CORES):
        out = res.results[c]
        ya = out["ya"]          # [2, P, KD, NT] bf16
        n = len(a_rows[c])
        yaf = np.concatenate([_ylay(ya[0]), _ylay(ya[1])], axis=0)[:n]
        y[a_rows[c]] += a_wts[c][:, None] * yaf
        nf = len(frag_rows[c])
        if nf:
            ybf = _ylay_b(out["yb"])[:nf]
            y[frag_rows[c]] += frag_wts[c][:, None] * ybf
        ssl = slice((c % 4) * SSLICE, (c % 4 + 1) * SSLICE)
        ysx = out["ys"]
        y[ssl] += np.concatenate([_ylay(ysx[0]), _ylay(ysx[1])], axis=0)
    for e, rows, wts in overflow_host:
        y[rows] += wts[:, None] * _ffn_host(x2d[rows], w1[e], w2[e], w3[e])

    return y.reshape(B, S, D)


# revision 6
# speedup vs baseline: 1.3325x; 1.0061x over previous
"""Expert-parallel MoE (Kimi/DeepSeek-V3 style sparse block) on 8 trn2 NeuronCores.

Strategy (v4 — balanced, bf16, DMA-streamed head):
  - Host computes the sigmoid gate + group-limited top-2 routing in numpy
    float64, then packs a *balanced* per-core schedule of five matmul chunks:
      [own-expert 512]x2  [shared 512]x2  [overflow-frag 128]
    Core c owns expert c's first 1024 tokens; per-expert overflow beyond 1024
    is split into <=128-token single-expert fragments bin-packed across the
    8 cores' fragment slots.  The shared FFN (SH=1024) is split into two
    F=512 halves; cores 0-3 run half 0 and cores 4-7 run half 1, each over a
    distinct 1024-token slice, summed on the host.
  - All weights/activations ship as bf16 in the exact SBUF layout; matmuls
    accumulate in fp32 PSUM; outputs return bf16 per-md for fast drain.
  - Head tuning: chunk 0's weight/x loads are split into 2-k-tile pieces and
    its loops run k-outer so the PE streams at DMA arrival rate; warm-up
    matmuls on a memset tile bridge the preamble so the PE never idles long
    enough to re-enter the HAM throttled state.
"""

from contextlib import ExitStack

import numpy as np
import ml_dtypes

import concourse.bacc as bacc
import concourse.tile as tile
import concourse.mybir as mybir
from concourse import bass_utils

# --- model dims (hardcoded per problem spec) ---
B, S, D = 2, 2048, 1024
T = B * S                 # 4096 tokens
E, F = 8, 512             # routed experts / expert intermediate
SH = 1024                 # shared intermediate
TOP_K, N_GROUP, TOPK_GROUP = 2, 4, 2
SCALE = 2.5

N_CORES = 8
P = 128                   # SBUF partitions
NT = 512                  # full token chunk (matmul moving free dim)
NB = 128                  # overflow-fragment chunk
KD = D // P               # 8 contraction tiles for D
KF = F // P               # 4 contraction tiles for F
ACAP = 2 * NT             # own-expert capacity per core (2 chunks)
SSLICE = 2 * NT           # shared tokens per core (2 chunks)

F32 = mybir.dt.float32
BF16 = mybir.dt.bfloat16
NPBF16 = ml_dtypes.bfloat16

_CACHE: dict = {}


def _emit(nc):
    """Per-core program: 2 own-expert chunks + 2 shared chunks + 1 fragment."""
    wa13 = nc.dram_tensor("wa13", [P, KD, 2 * F], BF16, kind="ExternalInput").ap()
    wa2 = nc.dram_tensor("wa2", [P, KF, D], BF16, kind="ExternalInput").ap()
    ws13 = nc.dram_tensor("ws13", [P, KD, 2 * F], BF16, kind="ExternalInput").ap()
    ws2 = nc.dram_tensor("ws2", [P, KF, D], BF16, kind="ExternalInput").ap()
    wb13 = nc.dram_tensor("wb13", [P, KD, 2 * F], BF16, kind="ExternalInput").ap()
    wb2 = nc.dram_tensor("wb2", [P, KF, D], BF16, kind="ExternalInput").ap()
    xa = nc.dram_tensor("xa", [2, P, KD, NT], BF16, kind="ExternalInput").ap()
    xs = nc.dram_tensor("xs", [2, P, KD, NT], BF16, kind="ExternalInput").ap()
    xb = nc.dram_tensor("xb", [P, KD, NB], BF16, kind="ExternalInput").ap()
    ya = nc.dram_tensor("ya", [2, KD, P, NT], BF16, kind="ExternalOutput").ap()
    ys = nc.dram_tensor("ys", [2, KD, P, NT], BF16, kind="ExternalOutput").ap()
    yb = nc.dram_tensor("yb", [KD, P, NB], BF16, kind="ExternalOutput").ap()

    silu = mybir.ActivationFunctionType.Silu

    with tile.TileContext(nc) as tc, ExitStack() as ctx:
        pool = ctx.enter_context(tc.tile_pool(name="sb", bufs=1))
        pspool = ctx.enter_context(tc.tile_pool(name="ps", bufs=1, space="PSUM"))

        ps_tags = [f"ps{i}" for i in range(8)]

        # --- HAM warm-up: matmuls on a zero tile bridge the DMA head ---
        warm = pool.tile([P, NT], BF16, name="warm", tag="warm")
        nc.any.memset(warm[:], 0)
        wps = pspool.tile([P, NT], F32, name="wps", tag=ps_tags[0])
        for _ in range(6):
            nc.tensor.matmul(wps[:], warm[:, 0:P], warm[:], start=True, stop=True)

        # --- loads: chunk 0 split into 2-k-tile pieces, w2A interleaved ---
        w13a_p, xa0_p = [], []

        def _lw13(i):
            t = pool.tile([P, 2, 2 * F], BF16, name=f"wa13p{i}", tag=f"wa13p{i}")
            nc.sync.dma_start(t[:], wa13[:, 2 * i:2 * i + 2])
            w13a_p.append(t)

        def _lx0(i):
            t = pool.tile([P, 2, NT], BF16, name=f"xa0p{i}", tag=f"xa0p{i}")
            nc.sync.dma_start(t[:], xa[0][:, 2 * i:2 * i + 2])
            xa0_p.append(t)

        _lw13(0); _lx0(0)
        _lw13(1); _lx0(1)
        wa2lo = pool.tile([P, 2, D], BF16, name="wa2lo", tag="wa2lo")
        nc.sync.dma_start(wa2lo[:], wa2[:, 0:2])
        _lw13(2); _lx0(2)
        wa2hi = pool.tile([P, 2, D], BF16, name="wa2hi", tag="wa2hi")
        nc.sync.dma_start(wa2hi[:], wa2[:, 2:4])
        _lw13(3); _lx0(3)
        xa1sb = pool.tile([P, KD, NT], BF16, name="xa1sb", tag="xa1")
        nc.sync.dma_start(xa1sb[:], xa[1])
        ws13sb = pool.tile([P, KD, 2 * F], BF16, name="ws13sb", tag="ws13")
        nc.sync.dma_start(ws13sb[:], ws13)
        xs0sb = pool.tile([P, KD, NT], BF16, name="xs0sb", tag="xs0")
        nc.sync.dma_start(xs0sb[:], xs[0])
        ws2sb = pool.tile([P, KF, D], BF16, name="ws2sb", tag="ws2")
        nc.sync.dma_start(ws2sb[:], ws2)
        xs1sb = pool.tile([P, KD, NT], BF16, name="xs1sb", tag="xs1")
        nc.sync.dma_start(xs1sb[:], xs[1])
        wb13sb = pool.tile([P, KD, 2 * F], BF16, name="wb13sb", tag="wb13")
        nc.sync.dma_start(wb13sb[:], wb13)
        xbsb = pool.tile([P, KD, NB], BF16, name="xbsb", tag="xb")
        nc.sync.dma_start(xbsb[:], xb)
        wb2sb = pool.tile([P, KF, D], BF16, name="wb2sb", tag="wb2")
        nc.sync.dma_start(wb2sb[:], wb2)

        def up_mfk(w13_at, x_at, n):
            """Up-projection, mf-outer (pipelines silu/mul into the stream)."""
            hts = []
            for mf in range(KF):
                h1 = pspool.tile([P, n], F32, name="h1ps", tag=ps_tags[2 * mf])
                for k in range(KD):
                    nc.tensor.matmul(
                        h1[:], w13_at(k)[:, mf * P:(mf + 1) * P], x_at(k),
                        start=(k == 0), stop=(k == KD - 1),
                    )
                h3 = pspool.tile([P, n], F32, name="h3ps", tag=ps_tags[2 * mf + 1])
                for k in range(KD):
                    nc.tensor.matmul(
                        h3[:], w13_at(k)[:, F + mf * P:F + (mf + 1) * P], x_at(k),
                        start=(k == 0), stop=(k == KD - 1),
                    )
                a = pool.tile([P, n], F32, name="asb", tag=f"a{mf % 2}")
                nc.scalar.activation(a[:], h1[:], silu)
                ht = pool.tile([P, n], BF16, name="htsb", tag=f"ht{mf}")
                nc.vector.tensor_mul(ht[:], a[:], h3[:])
                hts.append(ht)
            return hts

        def up_kmf(w13_at, x_at, n):
            """Up-projection, k-outer (streams at DMA arrival rate)."""
            h1s, h3s = [], []
            for k in range(KD):
                for mf in range(KF):
                    if k == 0:
                        h1s.append(pspool.tile([P, n], F32, name="h1ps",
                                               tag=ps_tags[mf]))
                    nc.tensor.matmul(
                        h1s[mf][:], w13_at(k)[:, mf * P:(mf + 1) * P], x_at(k),
                        start=(k == 0), stop=(k == KD - 1),
                    )
                for mf in range(KF):
                    if k == 0:
                        h3s.append(pspool.tile([P, n], F32, name="h3ps",
                                               tag=ps_tags[4 + mf]))
                    nc.tensor.matmul(
                        h3s[mf][:], w13_at(k)[:, F + mf * P:F + (mf + 1) * P],
                        x_at(k),
                        start=(k == 0), stop=(k == KD - 1),
                    )
            hts = []
            for mf in range(KF):
                a = pool.tile([P, n], F32, name="asb", tag=f"a{mf % 2}")
                nc.scalar.activation(a[:], h1s[mf][:], silu)
                ht = pool.tile([P, n], BF16, name="htsb", tag=f"ht{mf}")
                nc.vector.tensor_mul(ht[:], a[:], h3s[mf][:])
                hts.append(ht)
            return hts

        def down_md(w2_at, hts, n, ci, y_md):
            """Down-projection, md-outer (early per-md drain)."""
            osb = pool.tile([P, KD, n], BF16, name="osb", tag=f"osb{ci % 2}")
            for md in range(KD):
                yps = pspool.tile([P, n], F32, name="yps", tag=ps_tags[md])
                for kf in range(KF):
                    nc.tensor.matmul(
                        yps[:], w2_at(kf)[:, md * P:(md + 1) * P], hts[kf][:],
                        start=(kf == 0), stop=(kf == KF - 1),
                    )
                nc.vector.tensor_copy(osb[:, md, :], yps[:])
                nc.sync.dma_start(y_md(md), osb[:, md, :])

        def down_kf(w2_at, hts, n, ci, y_md):
            """Down-projection, kf-outer (streams at w2 DMA arrival rate)."""
            osb = pool.tile([P, KD, n], BF16, name="osb", tag=f"osb{ci % 2}")
            ypss = []
            for kf in range(KF):
                for md in range(KD):
                    if kf == 0:
                        ypss.append(pspool.tile([P, n], F32, name="yps",
                                                tag=ps_tags[md]))
                    nc.tensor.matmul(
                        ypss[md][:], w2_at(kf)[:, md * P:(md + 1) * P],
                        hts[kf][:],
                        start=(kf == 0), stop=(kf == KF - 1),
                    )
            for md in range(KD):
                nc.vector.tensor_copy(osb[:, md, :], ypss[md][:])
                nc.sync.dma_start(y_md(md), osb[:, md, :])

        # job 0: expert chunk 0 — fully DMA-streamed
        hts = up_kmf(lambda k: w13a_p[k // 2][:, k % 2],
                     lambda k: xa0_p[k // 2][:, k % 2], NT)
        down_kf(lambda kf: (wa2lo if kf < 2 else wa2hi)[:, kf % 2],
                hts, NT, 0, lambda md: ya[0][md])
        # job 1: expert chunk 1
        hts = up_mfk(lambda k: w13a_p[k // 2][:, k % 2],
                     lambda k: xa1sb[:, k], NT)
        down_md(lambda kf: (wa2lo if kf < 2 else wa2hi)[:, kf % 2],
                hts, NT, 1, lambda md: ya[1][md])
        # jobs 2-3: shared chunks
        hts = up_mfk(lambda k: ws13sb[:, k], lambda k: xs0sb[:, k], NT)
        down_md(lambda kf: ws2sb[:, kf], hts, NT, 2, lambda md: ys[0][md])
        hts = up_mfk(lambda k: ws13sb[:, k], lambda k: xs1sb[:, k], NT)
        down_md(lambda kf: ws2sb[:, kf], hts, NT, 3, lambda md: ys[1][md])
        # job 4: overflow fragment (small, short drain tail)
        hts = up_mfk(lambda k: wb13sb[:, k], lambda k: xbsb[:, k], NB)
        down_md(lambda kf: wb2sb[:, kf], hts, NB, 4, lambda md: yb[md])


def _get_nc():
    if "nc" not in _CACHE:
        nc = bacc.Bacc("TRN2", target_bir_lowering=False, debug=False,
                       num_devices=N_CORES)
        _emit(nc)
        nc.compile()
        _CACHE["nc"] = nc
    return _CACHE["nc"]


def _gate_numpy(x2d):
    """Replicates reference _moe_gate in float64 (routing-stable)."""
    xl = x2d.astype(np.float64)
    logits = xl @ _CACHE["gw64"].T
    scores = 1.0 / (1.0 + np.exp(-logits))
    sc = scores + _CACHE["gb64"][None, :]
    grp = sc.reshape(T, N_GROUP, E // N_GROUP)
    group_scores = np.sort(grp, axis=-1)[:, :, -2:].sum(-1)
    gidx = np.argsort(-group_scores, axis=-1, kind="stable")[:, :TOPK_GROUP]
    gmask = np.zeros((T, N_GROUP), bool)
    gmask[np.arange(T)[:, None], gidx] = True
    smask = np.repeat(gmask, E // N_GROUP, axis=1)
    tmp = np.where(smask, sc, 0.0)
    tidx = np.argsort(-tmp, axis=-1, kind="stable")[:, :TOP_K]
    tw = np.take_along_axis(scores, tidx, axis=1)
    tw = tw / (tw.sum(-1, keepdims=True) + 1e-20)
    return tidx, (tw * SCALE).astype(np.float32)


def _ffn_host(x, w1e, w2e, w3e):
    """Host fallback for tokens that don't fit the fragment slots (rare)."""
    h = x @ w1e.T
    h = (h / (1.0 + np.exp(-h))) * (x @ w3e.T)
    return h @ w2e.T


def _wlay_up(w):
    """[F, D] row-major -> [P, KD, F] bf16 (SBUF layout, d = k*P + p)."""
    return w.T.reshape(KD, P, F).transpose(1, 0, 2).astype(NPBF16)


def _wlay13(wg, wu):
    """gate/up [F, D] pair -> combined [P, KD, 2F] bf16."""
    return np.ascontiguousarray(
        np.concatenate([_wlay_up(wg), _wlay_up(wu)], axis=2))


def _wlay_down(w):
    """[D, F] row-major -> [P, KF, D] bf16 (SBUF layout, f = k*P + p)."""
    return np.ascontiguousarray(
        w.T.reshape(KF, P, D).transpose(1, 0, 2).astype(NPBF16))


def _xlay(xrows, nchunk, clen=NT):
    """[n<=nchunk*clen, D] tokens -> [nchunk, P, KD, clen] bf16, zero-padded."""
    n = xrows.shape[0]
    full = np.zeros((nchunk * clen, D), np.float32)
    full[:n] = xrows
    return np.ascontiguousarray(
        full.reshape(nchunk, clen, KD, P).transpose(0, 3, 2, 1).astype(NPBF16))


def _ylay(y):
    """[KD, P, n] bf16 -> [n, D] fp32."""
    kd, p, n = y.shape
    return y.astype(np.float32).reshape(kd * p, n).T


def kernel(hidden_states, gate_w, gate_bias, w1, w2, w3,
           shared_gate_w, shared_up_w, shared_down_w):
    hidden_states = np.ascontiguousarray(np.asarray(hidden_states, np.float32))
    gate_w = np.asarray(gate_w, np.float32)
    gate_bias = np.asarray(gate_bias, np.float32)
    w1 = np.asarray(w1, np.float32)
    w2 = np.asarray(w2, np.float32)
    w3 = np.asarray(w3, np.float32)
    shared_gate_w = np.asarray(shared_gate_w, np.float32)
    shared_up_w = np.asarray(shared_up_w, np.float32)
    shared_down_w = np.asarray(shared_down_w, np.float32)

    _CACHE["gw64"] = gate_w.astype(np.float64)
    _CACHE["gb64"] = gate_bias.astype(np.float64)

    x2d = hidden_states.reshape(T, D)
    tidx, tw = _gate_numpy(x2d)

    # --- per-expert token lists ---
    rows_e, wts_e = [], []
    for e in range(E):
        rows, slots = np.nonzero(tidx == e)
        rows_e.append(rows)
        wts_e.append(tw[rows, slots])

    # --- schedule: core c owns expert c's first ACAP tokens ---
    a_rows = [rows_e[c][:ACAP] for c in range(N_CORES)]
    a_wts = [wts_e[c][:ACAP] for c in range(N_CORES)]

    # overflow fragments: single-expert pieces of <=NB tokens, one per core slot
    frag_rows = [np.empty(0, np.int64)] * N_CORES
    frag_wts = [np.empty(0, np.float32)] * N_CORES
    frag_expert = [-1] * N_CORES
    overflow_host = []          # (expert, rows, wts) for host fallback
    slot = 0
    for e in range(E):
        left_r, left_w = rows_e[e][ACAP:], wts_e[e][ACAP:]
        pos = 0
        while pos < len(left_r):
            if slot >= N_CORES:
                overflow_host.append((e, left_r[pos:], left_w[pos:]))
                break
            take = min(NB, len(left_r) - pos)
            frag_rows[slot] = left_r[pos:pos + take]
            frag_wts[slot] = left_w[pos:pos + take]
            frag_expert[slot] = e
            slot += 1
            pos += take

    # --- build per-core inputs ---
    w13 = [_wlay13(w1[e], w3[e]) for e in range(E)]
    wdn = [_wlay_down(w2[e]) for e in range(E)]
    s13 = [_wlay13(shared_gate_w[h * F:(h + 1) * F],
                   shared_up_w[h * F:(h + 1) * F]) for h in range(2)]
    s2 = [_wlay_down(shared_down_w[:, h * F:(h + 1) * F]) for h in range(2)]

    in_maps = []
    for c in range(N_CORES):
        eb = frag_expert[c] if frag_expert[c] >= 0 else c
        h = c // 4
        ssl = slice((c % 4) * SSLICE, (c % 4 + 1) * SSLICE)
        in_maps.append({
            "wa13": w13[c], "wa2": wdn[c],
            "wb13": w13[eb], "wb2": wdn[eb],
            "ws13": s13[h], "ws2": s2[h],
            "xa": _xlay(x2d[a_rows[c]], 2),
            "xb": _xlay(x2d[frag_rows[c]], 1, NB)[0],
            "xs": _xlay(x2d[ssl], 2),
        })

    nc = _get_nc()
    res = bass_utils.run_bass_kernel_spmd(
        nc, in_maps, core_ids=list(range(N_CORES))
    )
    _CACHE["last_res"] = res

    y = np.zeros((T, D), np.float32)
    for c in range(N_CORES):
        out = res.results[c]
        ya = out["ya"]          # [2, KD, P, NT] bf16
        n = len(a_rows[c])
        yaf = np.concatenate([_ylay(ya[0]), _ylay(ya[1])], axis=0)[:n]
        y[a_rows[c]] += a_wts[c][:, None] * yaf
        nf = len(frag_rows[c])
        if nf:
            ybf = _ylay(out["yb"])[:nf]
            y[frag_rows[c]] += frag_wts[c][:, None] * ybf
        ssl = slice((c % 4) * SSLICE, (c % 4 + 1) * SSLICE)
        ysx = out["ys"]
        y[ssl] += np.concatenate([_ylay(ysx[0]), _ylay(ysx[1])], axis=0)
    for e, rows, wts in overflow_host:
        y[rows] += wts[:, None] * _ffn_host(x2d[rows], w1[e], w2[e], w3[e])

    return y.reshape(B, S, D)


# revision 9
# speedup vs baseline: 1.4730x; 1.1055x over previous
"""Expert-parallel MoE (Kimi/DeepSeek-V3 style sparse block) on 8 trn2 NeuronCores.

Strategy (v5 — balanced, bf16, DMA-lean):
  - Host computes the sigmoid gate + group-limited top-2 routing in numpy
    float64, then packs a balanced per-core schedule of four 512-token
    matmul chunks: [own-expert 512]x2  [shared 512]x2.
    Core c owns expert c's first 1024 tokens; the small per-expert overflow
    beyond 1024 (~2% of token-expert pairs for a balanced router) is
    evaluated on the host.  The shared FFN (SH=1024) is split into two F=512
    halves; cores 0-3 run half 0 and cores 4-7 run half 1, each over a
    distinct 1024-token slice, summed on the host.
  - All weights/activations ship as bf16 in the exact SBUF layout; matmuls
    accumulate in fp32 PSUM; outputs return bf16.
  - Overlap tuning: chunk 0's weight/x loads are split per contraction tile
    and its loops run k-outer so the PE streams at DMA arrival rate; warm-up
    matmuls on a memset tile bridge the preamble (HAM stays armed); output
    DMAs are batched 2-per-chunk (4 for the last chunk) to keep the Sync
    issue queue and the drain tail short.
"""

from contextlib import ExitStack

import numpy as np
import ml_dtypes

import concourse.bacc as bacc
import concourse.tile as tile
import concourse.mybir as mybir
from concourse import bass_utils

# --- model dims (hardcoded per problem spec) ---
B, S, D = 2, 2048, 1024
T = B * S                 # 4096 tokens
E, F = 8, 512             # routed experts / expert intermediate
SH = 1024                 # shared intermediate
TOP_K, N_GROUP, TOPK_GROUP = 2, 4, 2
SCALE = 2.5

N_CORES = 8
P = 128                   # SBUF partitions
NT = 512                  # token chunk (matmul moving free dim)
KD = D // P               # 8 contraction tiles for D
KF = F // P               # 4 contraction tiles for F
ACAP = 2 * NT             # own-expert capacity per core (2 chunks)
SSLICE = 2 * NT           # shared tokens per core (2 chunks)

F32 = mybir.dt.float32
BF16 = mybir.dt.bfloat16
NPBF16 = ml_dtypes.bfloat16

_CACHE: dict = {}


def _emit(nc):
    """Per-core program: 2 own-expert chunks + 2 shared chunks."""
    wa13 = nc.dram_tensor("wa13", [P, KD, 2 * F], BF16, kind="ExternalInput").ap()
    wa2 = nc.dram_tensor("wa2", [P, KF, D], BF16, kind="ExternalInput").ap()
    ws13 = nc.dram_tensor("ws13", [P, KD, 2 * F], BF16, kind="ExternalInput").ap()
    ws2 = nc.dram_tensor("ws2", [P, KF, D], BF16, kind="ExternalInput").ap()
    xa = nc.dram_tensor("xa", [2, P, KD, NT], BF16, kind="ExternalInput").ap()
    xs = nc.dram_tensor("xs", [2, P, KD, NT], BF16, kind="ExternalInput").ap()
    ya = nc.dram_tensor("ya", [2, P, KD, NT], BF16, kind="ExternalOutput").ap()
    ys = nc.dram_tensor("ys", [2, P, KD, NT], BF16, kind="ExternalOutput").ap()

    silu = mybir.ActivationFunctionType.Silu

    with tile.TileContext(nc) as tc, ExitStack() as ctx:
        pool = ctx.enter_context(tc.tile_pool(name="sb", bufs=1))
        pspool = ctx.enter_context(tc.tile_pool(name="ps", bufs=1, space="PSUM"))

        ps_tags = [f"ps{i}" for i in range(8)]

        # --- HAM warm-up: matmuls on a zero tile bridge the DMA head ---
        warm = pool.tile([P, NT], BF16, name="warm", tag="warm")
        nc.any.memset(warm[:], 0)
        wps = pspool.tile([P, NT], F32, name="wps", tag=ps_tags[0])
        for _ in range(4):
            nc.tensor.matmul(wps[:], warm[:, 0:P], warm[:], start=True, stop=True)

        # --- loads: chunk 0 split per k-tile so the PE streams immediately ---
        w13a_p, xa0_p = [], []
        for k in range(KD):
            t = pool.tile([P, 2 * F], BF16, name=f"wa13p{k}", tag=f"wa13p{k}")
            nc.sync.dma_start(t[:], wa13[:, k])
            w13a_p.append(t)
            t = pool.tile([P, NT], BF16, name=f"xa0p{k}", tag=f"xa0p{k}")
            nc.sync.dma_start(t[:], xa[0][:, k])
            xa0_p.append(t)
        wa2lo = pool.tile([P, 2, D], BF16, name="wa2lo", tag="wa2lo")
        nc.sync.dma_start(wa2lo[:], wa2[:, 0:2])
        wa2hi = pool.tile([P, 2, D], BF16, name="wa2hi", tag="wa2hi")
        nc.sync.dma_start(wa2hi[:], wa2[:, 2:4])
        xa1sb = pool.tile([P, KD, NT], BF16, name="xa1sb", tag="xa1")
        nc.sync.dma_start(xa1sb[:], xa[1])
        ws13sb = pool.tile([P, KD, 2 * F], BF16, name="ws13sb", tag="ws13")
        nc.sync.dma_start(ws13sb[:], ws13)
        xs0sb = pool.tile([P, KD, NT], BF16, name="xs0sb", tag="xs0")
        nc.sync.dma_start(xs0sb[:], xs[0])
        ws2sb = pool.tile([P, KF, D], BF16, name="ws2sb", tag="ws2")
        nc.sync.dma_start(ws2sb[:], ws2)
        xs1sb = pool.tile([P, KD, NT], BF16, name="xs1sb", tag="xs1")
        nc.sync.dma_start(xs1sb[:], xs[1])

        def up_mfk(w13_at, x_at, n):
            """Up-projection, mf-outer (pipelines silu/mul into the stream)."""
            hts = []
            for mf in range(KF):
                h1 = pspool.tile([P, n], F32, name="h1ps", tag=ps_tags[2 * mf])
                for k in range(KD):
                    nc.tensor.matmul(
                        h1[:], w13_at(k)[:, mf * P:(mf + 1) * P], x_at(k),
                        start=(k == 0), stop=(k == KD - 1),
                    )
                h3 = pspool.tile([P, n], F32, name="h3ps", tag=ps_tags[2 * mf + 1])
                for k in range(KD):
                    nc.tensor.matmul(
                        h3[:], w13_at(k)[:, F + mf * P:F + (mf + 1) * P], x_at(k),
                        start=(k == 0), stop=(k == KD - 1),
                    )
                a = pool.tile([P, n], F32, name="asb", tag=f"a{mf % 2}")
                nc.scalar.activation(a[:], h1[:], silu)
                ht = pool.tile([P, n], BF16, name="htsb", tag=f"ht{mf}")
                nc.vector.tensor_mul(ht[:], a[:], h3[:])
                hts.append(ht)
            return hts

        def up_kmf(w13_at, x_at, n):
            """Up-projection, k-outer (streams at DMA arrival rate)."""
            h1s, h3s = [], []
            for k in range(KD):
                for mf in range(KF):
                    if k == 0:
                        h1s.append(pspool.tile([P, n], F32, name="h1ps",
                                               tag=ps_tags[mf]))
                    nc.tensor.matmul(
                        h1s[mf][:], w13_at(k)[:, mf * P:(mf + 1) * P], x_at(k),
                        start=(k == 0), stop=(k == KD - 1),
                    )
                for mf in range(KF):
                    if k == 0:
                        h3s.append(pspool.tile([P, n], F32, name="h3ps",
                                               tag=ps_tags[4 + mf]))
                    nc.tensor.matmul(
                        h3s[mf][:], w13_at(k)[:, F + mf * P:F + (mf + 1) * P],
                        x_at(k),
                        start=(k == 0), stop=(k == KD - 1),
                    )
            hts = []
            for mf in range(KF):
                a = pool.tile([P, n], F32, name="asb", tag=f"a{mf % 2}")
                nc.scalar.activation(a[:], h1s[mf][:], silu)
                ht = pool.tile([P, n], BF16, name="htsb", tag=f"ht{mf}")
                nc.vector.tensor_mul(ht[:], a[:], h3s[mf][:])
                hts.append(ht)
            return hts

        def down_md(w2_at, hts, n, ci, y_ap, batches):
            """Down-projection, md-outer; output DMAs batched per `batches`."""
            osb = pool.tile([P, KD, n], BF16, name="osb", tag=f"osb{ci % 2}")
            for md in range(KD):
                yps = pspool.tile([P, n], F32, name="yps", tag=ps_tags[md])
                for kf in range(KF):
                    nc.tensor.matmul(
                        yps[:], w2_at(kf)[:, md * P:(md + 1) * P], hts[kf][:],
                        start=(kf == 0), stop=(kf == KF - 1),
                    )
                nc.vector.tensor_copy(osb[:, md, :], yps[:])
                if md + 1 in batches:
                    lo = batches[md + 1]
                    nc.sync.dma_start(y_ap[:, lo:md + 1, :], osb[:, lo:md + 1, :])

        def down_kf(w2_at, hts, n, ci, y_ap, batches):
            """Down-projection, kf-outer (streams at w2 DMA arrival rate)."""
            osb = pool.tile([P, KD, n], BF16, name="osb", tag=f"osb{ci % 2}")
            ypss = []
            for kf in range(KF):
                for md in range(KD):
                    if kf == 0:
                        ypss.append(pspool.tile([P, n], F32, name="yps",
                                                tag=ps_tags[md]))
                    nc.tensor.matmul(
                        ypss[md][:], w2_at(kf)[:, md * P:(md + 1) * P],
                        hts[kf][:],
                        start=(kf == 0), stop=(kf == KF - 1),
                    )
            for md in range(KD):
                nc.vector.tensor_copy(osb[:, md, :], ypss[md][:])
                if md + 1 in batches:
                    lo = batches[md + 1]
                    nc.sync.dma_start(y_ap[:, lo:md + 1, :], osb[:, lo:md + 1, :])

        half = {4: 0, 8: 4}
        quarter = {2: 0, 4: 2, 6: 4, 8: 6}

        # job 0: expert chunk 0 — fully DMA-streamed
        hts = up_kmf(lambda k: w13a_p[k], lambda k: xa0_p[k], NT)
        down_kf(lambda kf: (wa2lo if kf < 2 else wa2hi)[:, kf % 2],
                hts, NT, 0, ya[0], half)
        # job 1: expert chunk 1
        hts = up_mfk(lambda k: w13a_p[k], lambda k: xa1sb[:, k], NT)
        down_md(lambda kf: (wa2lo if kf < 2 else wa2hi)[:, kf % 2],
                hts, NT, 1, ya[1], half)
        # jobs 2-3: shared chunks
        hts = up_mfk(lambda k: ws13sb[:, k], lambda k: xs0sb[:, k], NT)
        down_md(lambda kf: ws2sb[:, kf], hts, NT, 2, ys[0], half)
        hts = up_mfk(lambda k: ws13sb[:, k], lambda k: xs1sb[:, k], NT)
        down_md(lambda kf: ws2sb[:, kf], hts, NT, 3, ys[1], quarter)


def _get_nc():
    if "nc" not in _CACHE:
        nc = bacc.Bacc("TRN2", target_bir_lowering=False, debug=False,
                       num_devices=N_CORES)
        _emit(nc)
        nc.compile()
        _CACHE["nc"] = nc
    return _CACHE["nc"]


def _gate_numpy(x2d):
    """Replicates reference _moe_gate in float64 (routing-stable)."""
    xl = x2d.astype(np.float64)
    logits = xl @ _CACHE["gw64"].T
    scores = 1.0 / (1.0 + np.exp(-logits))
    sc = scores + _CACHE["gb64"][None, :]
    grp = sc.reshape(T, N_GROUP, E // N_GROUP)
    group_scores = np.sort(grp, axis=-1)[:, :, -2:].sum(-1)
    gidx = np.argsort(-group_scores, axis=-1, kind="stable")[:, :TOPK_GROUP]
    gmask = np.zeros((T, N_GROUP), bool)
    gmask[np.arange(T)[:, None], gidx] = True
    smask = np.repeat(gmask, E // N_GROUP, axis=1)
    tmp = np.where(smask, sc, 0.0)
    tidx = np.argsort(-tmp, axis=-1, kind="stable")[:, :TOP_K]
    tw = np.take_along_axis(scores, tidx, axis=1)
    tw = tw / (tw.sum(-1, keepdims=True) + 1e-20)
    return tidx, (tw * SCALE).astype(np.float32)


def _ffn_host(x, w1e, w2e, w3e):
    """Host fallback for overflow tokens beyond the per-core capacity."""
    h = x @ w1e.T
    h = (h / (1.0 + np.exp(-h))) * (x @ w3e.T)
    return h @ w2e.T


def _wlay_up(w):
    """[F, D] row-major -> [P, KD, F] bf16 (SBUF layout, d = k*P + p)."""
    return w.T.reshape(KD, P, F).transpose(1, 0, 2).astype(NPBF16)


def _wlay13(wg, wu):
    """gate/up [F, D] pair -> combined [P, KD, 2F] bf16."""
    return np.ascontiguousarray(
        np.concatenate([_wlay_up(wg), _wlay_up(wu)], axis=2))


def _wlay_down(w):
    """[D, F] row-major -> [P, KF, D] bf16 (SBUF layout, f = k*P + p)."""
    return np.ascontiguousarray(
        w.T.reshape(KF, P, D).transpose(1, 0, 2).astype(NPBF16))


def _xlay(xrows, nchunk, clen=NT):
    """[n<=nchunk*clen, D] tokens -> [nchunk, P, KD, clen] bf16, zero-padded."""
    n = xrows.shape[0]
    full = np.zeros((nchunk * clen, D), np.float32)
    full[:n] = xrows
    return np.ascontiguousarray(
        full.reshape(nchunk, clen, KD, P).transpose(0, 3, 2, 1).astype(NPBF16))


def _ylay(y):
    """[P, KD, n] bf16 -> [n, D] fp32 (d = k*P + p)."""
    p, kd, n = y.shape
    return y.astype(np.float32).transpose(2, 1, 0).reshape(n, kd * p)


def kernel(hidden_states, gate_w, gate_bias, w1, w2, w3,
           shared_gate_w, shared_up_w, shared_down_w):
    hidden_states = np.ascontiguousarray(np.asarray(hidden_states, np.float32))
    gate_w = np.asarray(gate_w, np.float32)
    gate_bias = np.asarray(gate_bias, np.float32)
    w1 = np.asarray(w1, np.float32)
    w2 = np.asarray(w2, np.float32)
    w3 = np.asarray(w3, np.float32)
    shared_gate_w = np.asarray(shared_gate_w, np.float32)
    shared_up_w = np.asarray(shared_up_w, np.float32)
    shared_down_w = np.asarray(shared_down_w, np.float32)

    _CACHE["gw64"] = gate_w.astype(np.float64)
    _CACHE["gb64"] = gate_bias.astype(np.float64)

    x2d = hidden_states.reshape(T, D)
    tidx, tw = _gate_numpy(x2d)

    # --- per-expert token lists; overflow beyond ACAP goes to the host ---
    a_rows, a_wts, overflow_host = [], [], []
    for e in range(E):
        rows, slots = np.nonzero(tidx == e)
        wts = tw[rows, slots]
        a_rows.append(rows[:ACAP])
        a_wts.append(wts[:ACAP])
        if len(rows) > ACAP:
            overflow_host.append((e, rows[ACAP:], wts[ACAP:]))

    # --- build per-core inputs ---
    w13 = [_wlay13(w1[e], w3[e]) for e in range(E)]
    wdn = [_wlay_down(w2[e]) for e in range(E)]
    s13 = [_wlay13(shared_gate_w[h * F:(h + 1) * F],
                   shared_up_w[h * F:(h + 1) * F]) for h in range(2)]
    s2 = [_wlay_down(shared_down_w[:, h * F:(h + 1) * F]) for h in range(2)]

    in_maps = []
    for c in range(N_CORES):
        h = c // 4
        ssl = slice((c % 4) * SSLICE, (c % 4 + 1) * SSLICE)
        in_maps.append({
            "wa13": w13[c], "wa2": wdn[c],
            "ws13": s13[h], "ws2": s2[h],
            "xa": _xlay(x2d[a_rows[c]], 2),
            "xs": _xlay(x2d[ssl], 2),
        })

    nc = _get_nc()
    res = bass_utils.run_bass_kernel_spmd(
        nc, in_maps, core_ids=list(range(N_CORES))
    )
    _CACHE["last_res"] = res

    y = np.zeros((T, D), np.float32)
    for c in range(N_CORES):
        out = res.results[c]
        n = len(a_rows[c])
        yav = out["ya"]         # [2, P, KD, NT] bf16
        yaf = np.concatenate([_ylay(yav[0]), _ylay(yav[1])], axis=0)[:n]
        y[a_rows[c]] += a_wts[c][:, None] * yaf
        ssl = slice((c % 4) * SSLICE, (c % 4 + 1) * SSLICE)
        ysx = out["ys"]
        y[ssl] += np.concatenate([_ylay(ysx[0]), _ylay(ysx[1])], axis=0)
    for e, rows, wts in overflow_host:
        y[rows] += wts[:, None] * _ffn_host(x2d[rows], w1[e], w2[e], w3[e])

    return y.reshape(B, S, D)


# revision 10
# speedup vs baseline: 1.4822x; 1.0063x over previous
"""Expert-parallel MoE (Kimi/DeepSeek-V3 style sparse block) on 8 trn2 NeuronCores.

Strategy (v5 — balanced, bf16, DMA-lean):
  - Host computes the sigmoid gate + group-limited top-2 routing in numpy
    float64, then packs a balanced per-core schedule of four 512-token
    matmul chunks: [own-expert 512]x2  [shared 512]x2.
    Core c owns expert c's first 1024 tokens; the small per-expert overflow
    beyond 1024 (~2% of token-expert pairs for a balanced router) is
    evaluated on the host.  The shared FFN (SH=1024) is split into two F=512
    halves; cores 0-3 run half 0 and cores 4-7 run half 1, each over a
    distinct 1024-token slice, summed on the host.
  - All weights/activations ship as bf16 in the exact SBUF layout; matmuls
    accumulate in fp32 PSUM; outputs return bf16.
  - Overlap tuning: chunk 0's weight/x loads are split per contraction tile
    and its loops run k-outer so the PE streams at DMA arrival rate; warm-up
    matmuls on a memset tile bridge the preamble (HAM stays armed); output
    DMAs are batched 2-per-chunk (4 for the last chunk) to keep the Sync
    issue queue and the drain tail short.
"""

from contextlib import ExitStack

import numpy as np
import ml_dtypes

import concourse.bacc as bacc
import concourse.tile as tile
import concourse.mybir as mybir
from concourse import bass_utils

# --- model dims (hardcoded per problem spec) ---
B, S, D = 2, 2048, 1024
T = B * S                 # 4096 tokens
E, F = 8, 512             # routed experts / expert intermediate
SH = 1024                 # shared intermediate
TOP_K, N_GROUP, TOPK_GROUP = 2, 4, 2
SCALE = 2.5

N_CORES = 8
P = 128                   # SBUF partitions
NT = 512                  # token chunk (matmul moving free dim)
KD = D // P               # 8 contraction tiles for D
KF = F // P               # 4 contraction tiles for F
ACAP = 2 * NT             # own-expert capacity per core (2 chunks)
SSLICE = 2 * NT           # shared tokens per core (2 chunks)

F32 = mybir.dt.float32
BF16 = mybir.dt.bfloat16
NPBF16 = ml_dtypes.bfloat16

_CACHE: dict = {}


def _emit(nc):
    """Per-core program: 2 own-expert chunks + 2 shared chunks."""
    wa13 = nc.dram_tensor("wa13", [P, KD, 2 * F], BF16, kind="ExternalInput").ap()
    wa2 = nc.dram_tensor("wa2", [P, KF, D], BF16, kind="ExternalInput").ap()
    ws13 = nc.dram_tensor("ws13", [P, KD, 2 * F], BF16, kind="ExternalInput").ap()
    ws2 = nc.dram_tensor("ws2", [P, KF, D], BF16, kind="ExternalInput").ap()
    xa = nc.dram_tensor("xa", [2, P, KD, NT], BF16, kind="ExternalInput").ap()
    xs = nc.dram_tensor("xs", [2, P, KD, NT], BF16, kind="ExternalInput").ap()
    ya = nc.dram_tensor("ya", [2, P, KD, NT], BF16, kind="ExternalOutput").ap()
    ys = nc.dram_tensor("ys", [2, P, KD, NT], BF16, kind="ExternalOutput").ap()

    silu = mybir.ActivationFunctionType.Silu

    with tile.TileContext(nc) as tc, ExitStack() as ctx:
        pool = ctx.enter_context(tc.tile_pool(name="sb", bufs=1))
        pspool = ctx.enter_context(tc.tile_pool(name="ps", bufs=1, space="PSUM"))

        ps_tags = [f"ps{i}" for i in range(8)]

        # --- HAM warm-up: matmuls on a zero tile bridge the DMA head ---
        warm = pool.tile([P, NT], BF16, name="warm", tag="warm")
        nc.any.memset(warm[:], 0)
        wps = pspool.tile([P, NT], F32, name="wps", tag=ps_tags[0])
        for _ in range(4):
            nc.tensor.matmul(wps[:], warm[:, 0:P], warm[:], start=True, stop=True)

        # --- loads: chunk 0 split per k-tile so the PE streams immediately ---
        w13a_p, xa0_p = [], []
        w1p0 = pool.tile([P, F], BF16, name="w1p0", tag="w1p0")
        nc.sync.dma_start(w1p0[:], wa13[:, 0, 0:F])
        t = pool.tile([P, NT], BF16, name="xa0p0", tag="xa0p0")
        nc.sync.dma_start(t[:], xa[0][:, 0])
        xa0_p.append(t)
        w3p0 = pool.tile([P, F], BF16, name="w3p0", tag="w3p0")
        nc.sync.dma_start(w3p0[:], wa13[:, 0, F:2 * F])
        w13a_p.append(None)
        for k in range(1, KD):
            t = pool.tile([P, 2 * F], BF16, name=f"wa13p{k}", tag=f"wa13p{k}")
            nc.sync.dma_start(t[:], wa13[:, k])
            w13a_p.append(t)
            t = pool.tile([P, NT], BF16, name=f"xa0p{k}", tag=f"xa0p{k}")
            nc.sync.dma_start(t[:], xa[0][:, k])
            xa0_p.append(t)
        wa2lo = pool.tile([P, 2, D], BF16, name="wa2lo", tag="wa2lo")
        nc.sync.dma_start(wa2lo[:], wa2[:, 0:2])
        wa2hi = pool.tile([P, 2, D], BF16, name="wa2hi", tag="wa2hi")
        nc.sync.dma_start(wa2hi[:], wa2[:, 2:4])
        xa1sb = pool.tile([P, KD, NT], BF16, name="xa1sb", tag="xa1")
        nc.sync.dma_start(xa1sb[:], xa[1])
        ws13sb = pool.tile([P, KD, 2 * F], BF16, name="ws13sb", tag="ws13")
        nc.sync.dma_start(ws13sb[:], ws13)
        xs0sb = pool.tile([P, KD, NT], BF16, name="xs0sb", tag="xs0")
        nc.sync.dma_start(xs0sb[:], xs[0])
        ws2sb = pool.tile([P, KF, D], BF16, name="ws2sb", tag="ws2")
        nc.sync.dma_start(ws2sb[:], ws2)
        xs1sb = pool.tile([P, KD, NT], BF16, name="xs1sb", tag="xs1")
        nc.sync.dma_start(xs1sb[:], xs[1])

        def up_mfk(w1_at, w3_at, x_at, n):
            """Up-projection, mf-outer (pipelines silu/mul into the stream)."""
            hts = []
            for mf in range(KF):
                h1 = pspool.tile([P, n], F32, name="h1ps", tag=ps_tags[2 * mf])
                for k in range(KD):
                    nc.tensor.matmul(
                        h1[:], w1_at(k)[:, mf * P:(mf + 1) * P], x_at(k),
                        start=(k == 0), stop=(k == KD - 1),
                    )
                h3 = pspool.tile([P, n], F32, name="h3ps", tag=ps_tags[2 * mf + 1])
                for k in range(KD):
                    nc.tensor.matmul(
                        h3[:], w3_at(k)[:, mf * P:(mf + 1) * P], x_at(k),
                        start=(k == 0), stop=(k == KD - 1),
                    )
                a = pool.tile([P, n], F32, name="asb", tag=f"a{mf % 2}")
                nc.scalar.activation(a[:], h1[:], silu)
                ht = pool.tile([P, n], BF16, name="htsb", tag=f"ht{mf}")
                nc.vector.tensor_mul(ht[:], a[:], h3[:])
                hts.append(ht)
            return hts

        def up_kmf(w1_at, w3_at, x_at, n):
            """Up-projection, k-outer (streams at DMA arrival rate)."""
            h1s, h3s = [], []
            for k in range(KD):
                for mf in range(KF):
                    if k == 0:
                        h1s.append(pspool.tile([P, n], F32, name="h1ps",
                                               tag=ps_tags[mf]))
                    nc.tensor.matmul(
                        h1s[mf][:], w1_at(k)[:, mf * P:(mf + 1) * P], x_at(k),
                        start=(k == 0), stop=(k == KD - 1),
                    )
                for mf in range(KF):
                    if k == 0:
                        h3s.append(pspool.tile([P, n], F32, name="h3ps",
                                               tag=ps_tags[4 + mf]))
                    nc.tensor.matmul(
                        h3s[mf][:], w3_at(k)[:, mf * P:(mf + 1) * P],
                        x_at(k),
                        start=(k == 0), stop=(k == KD - 1),
                    )
            hts = []
            for mf in range(KF):
                a = pool.tile([P, n], F32, name="asb", tag=f"a{mf % 2}")
                nc.scalar.activation(a[:], h1s[mf][:], silu)
                ht = pool.tile([P, n], BF16, name="htsb", tag=f"ht{mf}")
                nc.vector.tensor_mul(ht[:], a[:], h3s[mf][:])
                hts.append(ht)
            return hts

        def down_md(w2_at, hts, n, ci, y_ap, batches):
            """Down-projection, md-outer; output DMAs batched per `batches`."""
            osb = pool.tile([P, KD, n], BF16, name="osb", tag=f"osb{ci % 2}")
            for md in range(KD):
                yps = pspool.tile([P, n], F32, name="yps", tag=ps_tags[md])
                for kf in range(KF):
                    nc.tensor.matmul(
                        yps[:], w2_at(kf)[:, md * P:(md + 1) * P], hts[kf][:],
                        start=(kf == 0), stop=(kf == KF - 1),
                    )
                nc.vector.tensor_copy(osb[:, md, :], yps[:])
                if md + 1 in batches:
                    lo = batches[md + 1]
                    nc.sync.dma_start(y_ap[:, lo:md + 1, :], osb[:, lo:md + 1, :])

        def down_kf(w2_at, hts, n, ci, y_ap, batches):
            """Down-projection, kf-outer (streams at w2 DMA arrival rate)."""
            osb = pool.tile([P, KD, n], BF16, name="osb", tag=f"osb{ci % 2}")
            ypss = []
            for kf in range(KF):
                for md in range(KD):
                    if kf == 0:
                        ypss.append(pspool.tile([P, n], F32, name="yps",
                                                tag=ps_tags[md]))
                    nc.tensor.matmul(
                        ypss[md][:], w2_at(kf)[:, md * P:(md + 1) * P],
                        hts[kf][:],
                        start=(kf == 0), stop=(kf == KF - 1),
                    )
            for md in range(KD):
                nc.vector.tensor_copy(osb[:, md, :], ypss[md][:])
                if md + 1 in batches:
                    lo = batches[md + 1]
                    nc.sync.dma_start(y_ap[:, lo:md + 1, :], osb[:, lo:md + 1, :])

        half = {4: 0, 8: 4}
        fine = {2: 0, 4: 2, 6: 4, 7: 6, 8: 7}

        wa1_at = lambda k: w1p0 if k == 0 else w13a_p[k][:, 0:F]
        wa3_at = lambda k: w3p0 if k == 0 else w13a_p[k][:, F:2 * F]

        # job 0: expert chunk 0 — fully DMA-streamed
        hts = up_kmf(wa1_at, wa3_at, lambda k: xa0_p[k], NT)
        down_kf(lambda kf: (wa2lo if kf < 2 else wa2hi)[:, kf % 2],
                hts, NT, 0, ya[0], half)
        # job 1: expert chunk 1
        hts = up_mfk(wa1_at, wa3_at, lambda k: xa1sb[:, k], NT)
        down_md(lambda kf: (wa2lo if kf < 2 else wa2hi)[:, kf % 2],
                hts, NT, 1, ya[1], half)
        # jobs 2-3: shared chunks
        hts = up_mfk(lambda k: ws13sb[:, k, 0:F], lambda k: ws13sb[:, k, F:2 * F],
                     lambda k: xs0sb[:, k], NT)
        down_md(lambda kf: ws2sb[:, kf], hts, NT, 2, ys[0], half)
        hts = up_mfk(lambda k: ws13sb[:, k, 0:F], lambda k: ws13sb[:, k, F:2 * F],
                     lambda k: xs1sb[:, k], NT)
        down_md(lambda kf: ws2sb[:, kf], hts, NT, 3, ys[1], fine)


def _get_nc():
    if "nc" not in _CACHE:
        nc = bacc.Bacc("TRN2", target_bir_lowering=False, debug=False,
                       num_devices=N_CORES)
        _emit(nc)
        nc.compile()
        _CACHE["nc"] = nc
    return _CACHE["nc"]


def _gate_numpy(x2d):
    """Replicates reference _moe_gate in float64 (routing-stable)."""
    xl = x2d.astype(np.float64)
    logits = xl @ _CACHE["gw64"].T
    scores = 1.0 / (1.0 + np.exp(-logits))
    sc = scores + _CACHE["gb64"][None, :]
    grp = sc.reshape(T, N_GROUP, E // N_GROUP)
    group_scores = np.sort(grp, axis=-1)[:, :, -2:].sum(-1)
    gidx = np.argsort(-group_scores, axis=-1, kind="stable")[:, :TOPK_GROUP]
    gmask = np.zeros((T, N_GROUP), bool)
    gmask[np.arange(T)[:, None], gidx] = True
    smask = np.repeat(gmask, E // N_GROUP, axis=1)
    tmp = np.where(smask, sc, 0.0)
    tidx = np.argsort(-tmp, axis=-1, kind="stable")[:, :TOP_K]
    tw = np.take_along_axis(scores, tidx, axis=1)
    tw = tw / (tw.sum(-1, keepdims=True) + 1e-20)
    return tidx, (tw * SCALE).astype(np.float32)


def _ffn_host(x, w1e, w2e, w3e):
    """Host fallback for overflow tokens beyond the per-core capacity."""
    h = x @ w1e.T
    h = (h / (1.0 + np.exp(-h))) * (x @ w3e.T)
    return h @ w2e.T


def _wlay_up(w):
    """[F, D] row-major -> [P, KD, F] bf16 (SBUF layout, d = k*P + p)."""
    return w.T.reshape(KD, P, F).transpose(1, 0, 2).astype(NPBF16)


def _wlay13(wg, wu):
    """gate/up [F, D] pair -> combined [P, KD, 2F] bf16."""
    return np.ascontiguousarray(
        np.concatenate([_wlay_up(wg), _wlay_up(wu)], axis=2))


def _wlay_down(w):
    """[D, F] row-major -> [P, KF, D] bf16 (SBUF layout, f = k*P + p)."""
    return np.ascontiguousarray(
        w.T.reshape(KF, P, D).transpose(1, 0, 2).astype(NPBF16))


def _xlay(xrows, nchunk, clen=NT):
    """[n<=nchunk*clen, D] tokens -> [nchunk, P, KD, clen] bf16, zero-padded."""
    n = xrows.shape[0]
    full = np.zeros((nchunk * clen, D), np.float32)
    full[:n] = xrows
    return np.ascontiguousarray(
        full.reshape(nchunk, clen, KD, P).transpose(0, 3, 2, 1).astype(NPBF16))


def _ylay(y):
    """[P, KD, n] bf16 -> [n, D] fp32 (d = k*P + p)."""
    p, kd, n = y.shape
    return y.astype(np.float32).transpose(2, 1, 0).reshape(n, kd * p)


def kernel(hidden_states, gate_w, gate_bias, w1, w2, w3,
           shared_gate_w, shared_up_w, shared_down_w):
    hidden_states = np.ascontiguousarray(np.asarray(hidden_states, np.float32))
    gate_w = np.asarray(gate_w, np.float32)
    gate_bias = np.asarray(gate_bias, np.float32)
    w1 = np.asarray(w1, np.float32)
    w2 = np.asarray(w2, np.float32)
    w3 = np.asarray(w3, np.float32)
    shared_gate_w = np.asarray(shared_gate_w, np.float32)
    shared_up_w = np.asarray(shared_up_w, np.float32)
    shared_down_w = np.asarray(shared_down_w, np.float32)

    _CACHE["gw64"] = gate_w.astype(np.float64)
    _CACHE["gb64"] = gate_bias.astype(np.float64)

    x2d = hidden_states.reshape(T, D)
    tidx, tw = _gate_numpy(x2d)

    # --- per-expert token lists; overflow beyond ACAP goes to the host ---
    a_rows, a_wts, overflow_host = [], [], []
    for e in range(E):
        rows, slots = np.nonzero(tidx == e)
        wts = tw[rows, slots]
        a_rows.append(rows[:ACAP])
        a_wts.append(wts[:ACAP])
        if len(rows) > ACAP:
            overflow_host.append((e, rows[ACAP:], wts[ACAP:]))

    # --- build per-core inputs ---
    w13 = [_wlay13(w1[e], w3[e]) for e in range(E)]
    wdn = [_wlay_down(w2[e]) for e in range(E)]
    s13 = [_wlay13(shared_gate_w[h * F:(h + 1) * F],
                   shared_up_w[h * F:(h + 1) * F]) for h in range(2)]
    s2 = [_wlay_down(shared_down_w[:, h * F:(h + 1) * F]) for h in range(2)]

    in_maps = []
    for c in range(N_CORES):
        h = c // 4
        ssl = slice((c % 4) * SSLICE, (c % 4 + 1) * SSLICE)
        in_maps.append({
            "wa13": w13[c], "wa2": wdn[c],
            "ws13": s13[h], "ws2": s2[h],
            "xa": _xlay(x2d[a_rows[c]], 2),
            "xs": _xlay(x2d[ssl], 2),
        })

    nc = _get_nc()
    res = bass_utils.run_bass_kernel_spmd(
        nc, in_maps, core_ids=list(range(N_CORES))
    )
    _CACHE["last_res"] = res

    y = np.zeros((T, D), np.float32)
    for c in range(N_CORES):
        out = res.results[c]
        n = len(a_rows[c])
        yav = out["ya"]         # [2, P, KD, NT] bf16
        yaf = np.concatenate([_ylay(yav[0]), _ylay(yav[1])], axis=0)[:n]
        y[a_rows[c]] += a_wts[c][:, None] * yaf
        ssl = slice((c % 4) * SSLICE, (c % 4 + 1) * SSLICE)
        ysx = out["ys"]
        y[ssl] += np.concatenate([_ylay(ysx[0]), _ylay(ysx[1])], axis=0)
    for e, rows, wts in overflow_host:
        y[rows] += wts[:, None] * _ffn_host(x2d[rows], w1[e], w2[e], w3[e])

    return y.reshape(B, S, D)
